# revision 16
# baseline (speedup 1.0000x reference)
"""ABCNN3 distributed Bass kernel for 8 TRN2 NeuronCores.

Key algebraic insight: the reference only consumes *global average pools* of
the conv outputs, and conv + avg-pool are linear.  So for each sentence the
whole conv stack collapses to functions of (a) the per-channel sum over time
S[c] = sum_t x[c,t] and (b) the first/last 3 time-columns (pool edge
corrections).  The kernel is therefore a memory-bound row-sum over the big
[1024, 512, 30] tensor followed by tiny 30x30 matmuls, cosine sims, and a
2-class MLP.  Cosine similarity is scale-invariant, so all pooled vectors are
kept unnormalized; bias terms are folded into the matmuls via constant rows
(515/131 and 512/128 length constants).  The 2-class log-softmax reduces to
sigmoid/log-sigmoid of the logit difference.

Sharding: data-parallel over the sentence axis N (128 sentences/core),
question + parameters replicated.  Each core emits its 128 softmax outputs
plus its partial NLL sum; the host gather concatenates the former and adds
the latter (the unshard step for a mean-reduced scalar).
"""

import os
import sys

import numpy as np

for _p in ("/opt/trn_rl_repo",):
    if _p not in sys.path and os.path.isdir(_p):
        sys.path.insert(0, _p)

import concourse.bacc as bacc
import concourse.mybir as mybir
import concourse.tile as tile
from concourse.bass_utils import run_bass_kernel_spmd

F32 = mybir.dt.float32
U32 = mybir.dt.uint32
AX = mybir.AxisListType
ALU = mybir.AluOpType
ACTF = mybir.ActivationFunctionType

NCORES = 8
N, S, C, Q = 1024, 512, 30, 128
NS = N // NCORES          # sentences per core
CH = 8                    # DMA/reduce chunks along t
TCH = S // CH             # t per chunk
CW = TCH * C              # floats per chunk per sentence
W = NS + 1                # matmul width: 128 sentence cols + 1 question col
QC = NS                   # question column index

_CACHE = {}


def _build():
    nc = bacc.Bacc("TRN2", target_bir_lowering=False, debug=False,
                   enable_asserts=False, num_devices=NCORES)

    xs = nc.dram_tensor("xs", [NS, S * C], F32, kind="ExternalInput").ap()
    qe = nc.dram_tensor("qe", [Q, C], F32, kind="ExternalInput").ap()
    w1 = nc.dram_tensor("w1", [C, C * 4], F32, kind="ExternalInput").ap()
    b1 = nc.dram_tensor("b1", [1, C], F32, kind="ExternalInput").ap()
    w2 = nc.dram_tensor("w2", [C, C * 4], F32, kind="ExternalInput").ap()
    b2 = nc.dram_tensor("b2", [1, C], F32, kind="ExternalInput").ap()
    aux = nc.dram_tensor("aux", [16, W + 1], F32, kind="ExternalInput").ap()
    qbnd = nc.dram_tensor("qbnd", [94, 2], F32, kind="ExternalInput").ap()
    slab = nc.dram_tensor("slab", [NS, 1], F32, kind="ExternalInput").ap()
    wall = nc.dram_tensor("wall", [14, 2], F32, kind="ExternalInput").ap()
    iden = nc.dram_tensor("iden", [128, 128], F32, kind="ExternalInput").ap()
    out = nc.dram_tensor("out", [1, W], F32, kind="ExternalOutput").ap()

    with tile.TileContext(nc) as tc:
        with (
            tc.tile_pool(name="sb", bufs=1) as sb,
            tc.tile_pool(name="ps", bufs=8, space="PSUM") as ps,
        ):
            # ---------------- persistent SBUF tiles ----------------
            xt = [sb.tile([NS, CW], F32, tag=f"x{i}", name=f"x{i}")
                  for i in range(CH)]
            acc = sb.tile([NS, CW], F32, tag="acc", name="acc")
            q_sb = sb.tile([Q, C], F32, tag="q_sb", name="q_sb")
            w1_sb = sb.tile([C, C * 4], F32, tag="w1_sb", name="w1_sb")
            w2_sb = sb.tile([C, C * 4], F32, tag="w2_sb", name="w2_sb")
            slab_sb = sb.tile([NS, 1], F32, tag="slab_sb", name="slab_sb")
            iden_sb = sb.tile([128, 128], F32, tag="iden_sb", name="iden_sb")
            mask_sb = sb.tile([1, 1], F32, tag="mask_sb", name="mask_sb")
            ones_sb = sb.tile([128, 1], F32, tag="ones_sb", name="ones_sb")

            # stacked rhs/lhs for the collapsed conv matmuls.
            # SF rows: 0-29 S^T | 30 len2 | 31 len1 | 32-61 x(0) | 62-63 0 |
            #          64-93 x(1) | 94-95 0 | 96-125 x(2)
            SF = sb.tile([126, W], F32, tag="SF", name="SF")
            # Bstk rows: 0-29 x(L-3) | 30-31 0 | 32-61 x(L-2) | 62-63 0 |
            #            64-93 x(L-1)
            Bstk = sb.tile([94, W], F32, tag="Bstk", name="Bstk")
            rhs_Z = sb.tile([32, W], F32, tag="rhs_Z", name="rhs_Z")
            lhsT_SF = sb.tile([126, C], F32, tag="lhsT_SF", name="lhsT_SF")
            lhsT_B = sb.tile([94, C], F32, tag="lhsT_B", name="lhsT_B")
            lhsT1x = sb.tile([32, C], F32, tag="lhsT1x", name="lhsT1x")
            lhsT2x = sb.tile([32, C], F32, tag="lhsT2x", name="lhsT2x")
            # post-conv stack: 0-29 S | 32-61 sg1 | 64-93 sg2
            stk = sb.tile([94, W], F32, tag="stk", name="stk")
            sqk = sb.tile([94, W], F32, tag="sqk", name="sqk")
            lhsT_q = sb.tile([94, 4], F32, tag="lhsT_q", name="lhsT_q")
            mask4 = sb.tile([94, 4], F32, tag="mask4", name="mask4")
            sdif = sb.tile([4, 1], F32, tag="sdif", name="sdif")
            G_sb = sb.tile([4, 4], F32, tag="G_sb", name="G_sb")
            lwr0 = sb.tile([1, 2], F32, tag="lwr0", name="lwr0")
            wall_sb = sb.tile([14, 2], F32, tag="wall_sb", name="wall_sb")
            # rows 0-3 get the sims (DVE), rows 4-13 gaf+ones (DMA)
            mlp_all = sb.tile([14, W], F32, tag="mlp_all", name="mlp_all")
            warm = sb.tile([1, 1], F32, tag="warm", name="warm")
            outst = sb.tile([1, W], F32, tag="outst", name="outst")
            magic = sb.tile([4, W], U32, tag="magic", name="magic")

            # ---------------- zero/const fills ----------------
            for t in (SF, Bstk, rhs_Z, lhsT_SF, lhsT_B, lhsT1x, lhsT2x,
                      stk, lhsT_q, mask4):
                nc.gpsimd.memset(t[:], 0.0)
            nc.gpsimd.memset(ones_sb[:], 1.0)
            nc.gpsimd.memset(magic[:], 0x5f3759df)

            # ---------------- input DMAs ----------------
            # big x chunks: 4 on each HWDGE ring; all small DMAs go on the
            # sync ring so the scalar engine stays free for compute
            nc.sync.dma_start(iden_sb[:], iden)
            nc.sync.dma_start(w1_sb[:], w1)
            nc.sync.dma_start(w2_sb[:], w2)
            nc.sync.dma_start(q_sb[:], qe)
            for i in range(CH):
                eng = nc.sync if i % 2 == 0 else nc.scalar
                eng.dma_start(xt[i][:], xs[:, i * CW:(i + 1) * CW])
            nc.sync.dma_start(slab_sb[:], slab)
            nc.sync.dma_start(mask_sb[:], aux[2:3, 0:1])
            nc.sync.dma_start(SF[30:32, 0:W], aux[0:2, 0:W])   # len2, len1
            nc.sync.dma_start(rhs_Z[31:32, 0:W], aux[1:2, 0:W])  # len1
            nc.sync.dma_start(lhsT_SF[30:31, 0:C], b1)
            nc.sync.dma_start(lhsT1x[31:32, 0:C], b1)
            nc.sync.dma_start(lhsT2x[31:32, 0:C], b2)
            # question boundary columns (gap rows pre-zeroed on host)
            nc.sync.dma_start(SF[32:126, QC:QC + 1], qbnd[:, 0:1])
            nc.sync.dma_start(Bstk[0:94, QC:QC + 1], qbnd[:, 1:2])
            nc.sync.dma_start(mlp_all[4:14, 0:NS], aux[3:13, 0:NS])
            nc.sync.dma_start(sdif[:], aux[13:14, 0:4].rearrange("r c -> c r"))
            nc.sync.dma_start(
                G_sb[:], aux[14:15, 0:16].rearrange("a (p j) -> (a p) j", p=4))
            nc.sync.dma_start(lwr0[:], aux[15:16, 0:2])
            nc.sync.dma_start(wall_sb[:], wall)

            # sigmoid table warmup: loads the set early so the tail sigmoid
            # runs without an ACT_TABLE_LOAD (copies are in every set)
            nc.scalar.activation(warm[:], ones_sb[0:1, 0:1], ACTF.Sigmoid)

            # mask4: block indicator columns; col3 reuses the S block
            for r, base in ((0, 0), (1, 32), (2, 64), (3, 0)):
                nc.vector.tensor_copy(mask4[base:base + C, r:r + 1],
                                      ones_sb[base:base + C, 0:1])

            # ---------------- weight prep (all tiny, overlaps DMA) -------
            def wk1(k):
                return w1_sb[:, k:C * 4:4]

            def wk2(k):
                return w2_sb[:, k:C * 4:4]

            wk = wk1
            W1s = sb.tile([C, C], F32, tag="W1s", name="W1s")
            W2s = sb.tile([C, C], F32, tag="W2s", name="W2s")
            nc.vector.tensor_reduce(
                W1s[:], w1_sb[:].rearrange("p (i k) -> p i k", k=4),
                axis=AX.X, op=ALU.add)
            nc.vector.tensor_reduce(
                W2s[:], w2_sb[:].rearrange("p (i k) -> p i k", k=4),
                axis=AX.X, op=ALU.add)

            t1 = sb.tile([C, C], F32, tag="t1", name="t1")
            t2 = sb.tile([C, C], F32, tag="t2", name="t2")
            NF0 = sb.tile([C, C], F32, tag="NF0", name="NF0")
            NF1 = sb.tile([C, C], F32, tag="NF1", name="NF1")
            NF2 = sb.tile([C, C], F32, tag="NF2", name="NF2")
            NB0 = sb.tile([C, C], F32, tag="NB0", name="NB0")
            NB1 = sb.tile([C, C], F32, tag="NB1", name="NB1")
            NB2 = sb.tile([C, C], F32, tag="NB2", name="NB2")

            # front coeffs (negated): NF0 -> x[0], NF1 -> x[1], NF2 -> x[2]
            nc.vector.tensor_scalar(t1[:], wk(2), 2.0, None, ALU.mult)
            nc.vector.tensor_add(t1[:], t1[:], wk(1))
            nc.vector.tensor_scalar(t2[:], wk(3), 3.0, None, ALU.mult)
            nc.vector.tensor_add(t1[:], t1[:], t2[:])
            nc.vector.tensor_scalar(NF0[:], t1[:], -0.25, None, ALU.mult)
            nc.vector.tensor_scalar(t1[:], wk(3), 2.0, None, ALU.mult)
            nc.vector.tensor_add(t1[:], t1[:], wk(2))
            nc.vector.tensor_scalar(NF1[:], t1[:], -0.25, None, ALU.mult)
            nc.vector.tensor_scalar(NF2[:], wk(3), -0.25, None, ALU.mult)
            # back coeffs (negated): NB0 -> x[L-3], NB1 -> x[L-2], NB2 -> x[L-1]
            nc.vector.tensor_scalar(NB0[:], wk(0), -0.25, None, ALU.mult)
            nc.vector.tensor_scalar(t1[:], wk(0), 2.0, None, ALU.mult)
            nc.vector.tensor_add(t1[:], t1[:], wk(1))
            nc.vector.tensor_scalar(NB1[:], t1[:], -0.25, None, ALU.mult)
            nc.vector.tensor_scalar(t1[:], wk(0), 3.0, None, ALU.mult)
            nc.vector.tensor_scalar(t2[:], wk(1), 2.0, None, ALU.mult)
            nc.vector.tensor_add(t1[:], t1[:], t2[:])
            nc.vector.tensor_add(t1[:], t1[:], wk(2))
            nc.vector.tensor_scalar(NB2[:], t1[:], -0.25, None, ALU.mult)

            id30 = iden_sb[0:C, 0:C]
            # small-weight transposes via regular matmuls (out = src^T) so
            # they can land at quadrant bases of one psum tile
            psL = ps.tile([126, C], F32, tag="ps", name="psL")
            for src, base in ((W1s, 0), (NF0, 32), (NF1, 64), (NF2, 96)):
                nc.tensor.matmul(psL[base:base + C, 0:C], src[:], id30,
                                 tile_position=(0, base))
            for base in (0, 32, 64, 96):
                nc.scalar.copy(lhsT_SF[base:base + C, 0:C],
                               psL[base:base + C, 0:C])
            nc.scalar.copy(lhsT1x[0:C, 0:C], psL[0:C, 0:C])
            psLB = ps.tile([94, C], F32, tag="ps", name="psLB")
            for src, base in ((NB0, 0), (NB1, 32), (NB2, 64)):
                nc.tensor.matmul(psLB[base:base + C, 0:C], src[:], id30,
                                 tile_position=(0, base))
            for base in (0, 32, 64):
                nc.scalar.copy(lhsT_B[base:base + C, 0:C],
                               psLB[base:base + C, 0:C])
            psW2 = ps.tile([C, C], F32, tag="ps", name="psW2")
            nc.tensor.matmul(psW2[:], W2s[:], id30)
            nc.scalar.copy(lhsT2x[0:C, 0:C], psW2[:])

            # x boundary transposes (need only chunks 0 and 7)
            psSF = ps.tile([126, W], F32, tag="ps", name="psSF")
            for t, base in enumerate((32, 64, 96)):
                nc.tensor.matmul(psSF[base:base + C, 0:NS],
                                 xt[0][:, t * C:(t + 1) * C],
                                 iden_sb[:, :], tile_position=(0, base))
                nc.scalar.copy(SF[base:base + C, 0:NS],
                               psSF[base:base + C, 0:NS])
            psB = ps.tile([94, NS], F32, tag="ps", name="psB")
            for t, base in enumerate((0, 32, 64)):
                off = CW - 90 + t * C
                nc.tensor.matmul(psB[base:base + C, 0:NS],
                                 xt[CH - 1][:, off:off + C],
                                 iden_sb[:, :], tile_position=(0, base))
                nc.scalar.copy(Bstk[base:base + C, 0:NS],
                               psB[base:base + C, 0:NS])

            # ---------------- big reduction: S[n,c] = sum_t x ------------
            nc.vector.tensor_add(acc[:], xt[0][:], xt[1][:])
            for i in range(2, CH):
                nc.vector.tensor_add(acc[:], acc[:], xt[i][:])
            w_ = CW
            while w_ > C:
                h = w_ // 2
                nc.vector.tensor_add(acc[:, 0:h], acc[:, 0:h], acc[:, h:w_])
                w_ = h
            # S = acc[:, 0:30]  (per-sentence channel sums)

            # S^T into SF rows 0-29 (+ question column Sq)
            nc.tensor.transpose(psSF[0:C, 0:NS], acc[:, 0:C], iden_sb[:, :])
            nc.tensor.matmul(psSF[0:C, QC:QC + 1], q_sb[:], ones_sb[:, :])
            nc.scalar.copy(SF[0:C, 0:W], psSF[0:C, 0:W])

            # ---------------- collapsed conv matmuls ----------------
            # Z (window-pooled conv1 channel sums), sentences + question
            psZ = ps.tile([C, W], F32, tag="ps", name="psZ")
            nc.tensor.matmul(psZ[:], lhsT_SF[0:126, 0:C], SF[0:126, 0:W],
                             start=True, stop=False)
            nc.tensor.matmul(psZ[:], lhsT_B[0:94, 0:C], Bstk[0:94, 0:W],
                             start=False, stop=True)
            nc.scalar.copy(rhs_Z[0:C, 0:W], psZ[:])

            # post-conv stack in one psum tile: sg1@32, sg2@64
            psStk = ps.tile([94, W], F32, tag="ps", name="psStk")
            nc.tensor.matmul(psStk[32:62, 0:W], lhsT1x[0:32, 0:C],
                             SF[0:32, 0:W], tile_position=(0, 32))
            nc.tensor.matmul(psStk[64:94, 0:W], lhsT2x[0:32, 0:C],
                             rhs_Z[0:32, 0:W], tile_position=(0, 64))
            # qg2 again at base 0 (for the qg2 . S similarity column)
            psQ2 = ps.tile([C, 1], F32, tag="ps", name="psQ2")
            nc.tensor.matmul(psQ2[:], lhsT2x[0:32, 0:C],
                             rhs_Z[0:32, QC:QC + 1])
            nc.vector.tensor_copy(stk[0:C, 0:W], SF[0:C, 0:W])
            nc.scalar.copy(stk[32:62, 0:W], psStk[32:62, 0:W])
            nc.scalar.copy(stk[64:94, 0:W], psStk[64:94, 0:W])

            # squares for the norms; q-vector columns for the dots
            nc.vector.tensor_mul(sqk[:], stk[:], stk[:])
            for r, base in ((0, 0), (1, 32), (2, 64)):
                nc.vector.tensor_copy(lhsT_q[base:base + C, r:r + 1],
                                      stk[base:base + C, QC:QC + 1])
            nc.vector.tensor_copy(lhsT_q[0:C, 3:4], psQ2[:])

            # dots4 rows: 0 = Sq.S (simA), 1 = qg1.sg1, 2 = qg2.sg2,
            #             3 = qg2.S (simB);  nrm4 rows: |S|,|sg1|,|sg2|,|S|^2
            dots4 = ps.tile([4, W], F32, tag="ps", name="dots4")
            nc.tensor.matmul(dots4[:], lhsT_q[0:94, 0:4], stk[0:94, 0:W])
            nrm4 = ps.tile([4, W], F32, tag="ps", name="nrm4")
            nc.tensor.matmul(nrm4[:], mask4[0:94, 0:4], sqk[0:94, 0:W])

            # q-side squared norms: [|Sq|^2,|qg1|^2,|qg2|^2,|qg2|^2] via G
            qncol = sb.tile([4, 1], F32, tag="qncol", name="qncol")
            nc.scalar.copy(qncol[:], nrm4[0:4, QC:QC + 1])
            psQn = ps.tile([4, 1], F32, tag="ps", name="psQn")
            nc.tensor.matmul(psQn[:], G_sb[:], qncol[:])
            qn = sb.tile([4, 1], F32, tag="qn", name="qn")
            nc.scalar.copy(qn[:], psQn[:])
            ppack = sb.tile([4, W], F32, tag="ppack", name="ppack")
            nc.vector.tensor_scalar(ppack[:], nrm4[0:4, 0:W], qn[:],
                                    None, ALU.mult)

            # rsqrt(ppack) on DVE: magic-number seed + 2 Newton steps
            ish = sb.tile([4, W], U32, tag="ish", name="ish")
            nc.vector.tensor_scalar(ish[:], ppack[:].bitcast(U32), 1,
                                    None, ALU.logical_shift_right)
            y0 = sb.tile([4, W], F32, tag="y0", name="y0")
            nc.vector.tensor_sub(y0[:].bitcast(U32), magic[:], ish[:])
            na = sb.tile([4, W], F32, tag="na", name="na")
            nb = sb.tile([4, W], F32, tag="nb", name="nb")
            nc.vector.tensor_mul(na[:], y0[:], y0[:])
            nc.vector.tensor_mul(nb[:], na[:], ppack[:])
            nc.vector.tensor_scalar(nb[:], nb[:], -0.5, 1.5, ALU.mult,
                                    ALU.add)
            rs = sb.tile([4, W], F32, tag="rs", name="rs")
            nc.vector.tensor_mul(rs[:], y0[:], nb[:])
            # second Newton step for accuracy (cheap, [4,W] only)
            nc.vector.tensor_mul(na[:], rs[:], rs[:])
            nc.vector.tensor_mul(nb[:], na[:], ppack[:])
            nc.vector.tensor_scalar(nb[:], nb[:], -0.5, 1.5, ALU.mult,
                                    ALU.add)
            nc.vector.tensor_mul(rs[:], rs[:], nb[:])

            # sims straight into the MLP lhsT rows 0-3
            nc.vector.tensor_mul(mlp_all[0:4, 0:W], dots4[0:4, 0:W], rs[:])

            # sim1 = simB, except sentence 0 of core 0 -> simA.
            dif_ps = ps.tile([1, 1], F32, tag="ps", name="dif_ps")
            nc.tensor.matmul(dif_ps[:], sdif[0:4, 0:1], mlp_all[0:4, 0:1])
            md = sb.tile([1, 1], F32, tag="md", name="md")
            nc.vector.tensor_mul(md[:], dif_ps[:], mask_sb[:])
            lcorr = sb.tile([1, 2], F32, tag="lcorr", name="lcorr")
            nc.vector.tensor_scalar(lcorr[:], lwr0[:], md[:], None, ALU.mult)

            # ---------------- MLP + 2-class softmax + outputs ------------
            logits = ps.tile([NS, 2], F32, tag="ps", name="logits")
            nc.tensor.matmul(logits[:], mlp_all[0:14, 0:NS], wall_sb[:])
            nc.vector.tensor_add(logits[0:1, 0:2], logits[0:1, 0:2],
                                 lcorr[:])
            lg_sb = sb.tile([NS, 2], F32, tag="lg_sb", name="lg_sb")
            nc.vector.tensor_copy(lg_sb[:], logits[:])
            dz = sb.tile([NS, 1], F32, tag="dz", name="dz")
            nc.vector.tensor_sub(dz[:], lg_sb[:, 1:2], lg_sb[:, 0:1])
            sdz = sb.tile([NS, 1], F32, tag="sdz", name="sdz")
            nc.vector.tensor_mul(sdz[:], dz[:], slab_sb[:])
            # emit = sigmoid(dz); picked log-prob = ln(sigmoid(sdz))
            em = sb.tile([NS, 1], F32, tag="em", name="em")
            nc.scalar.activation(em[:], dz[:], ACTF.Sigmoid)
            psel = sb.tile([NS, 1], F32, tag="psel", name="psel")
            nc.scalar.activation(psel[:], sdz[:], ACTF.Sigmoid)
            pick = sb.tile([NS, 1], F32, tag="pick", name="pick")
            nc.scalar.activation(pick[:], psel[:], ACTF.Ln)
            costp = ps.tile([1, 1], F32, tag="ps", name="costp")
            nc.tensor.matmul(costp[:], ones_sb[:, :], pick[:])
            # partial cost (already scaled); host sums the 8 partials
            nc.vector.tensor_scalar(outst[0:1, NS:NS + 1], costp[:],
                                    -1.0 / N, None, ALU.mult)
            psE = ps.tile([1, NS], F32, tag="ps", name="psE")
            nc.tensor.transpose(psE[:], em[:], iden_sb[:, :])
            nc.vector.tensor_copy(outst[0:1, 0:NS], psE[:])
            nc.sync.dma_start(out[0:1, 0:W], outst[:])

    nc.compile()
    return nc


def _get_nc():
    if "nc" not in _CACHE:
        _CACHE["nc"] = _build()
    return _CACHE["nc"]


def _make_in_maps(question_embeds, sents_embeds, sents_gaf, sents_labels,
                  conv1_w, conv1_b, conv2_w, conv2_b, lin_w, lin_b):
    f32 = lambda a: np.ascontiguousarray(np.asarray(a), dtype=np.float32)
    q = f32(question_embeds)                       # [128, 30]
    x = f32(sents_embeds).reshape(N, S * C)        # [1024, 15360]
    gaf = f32(sents_gaf)                           # [1024, 9]
    lab = f32(np.asarray(sents_labels))            # [1024]
    w1m = f32(conv1_w).reshape(C, C * 4)
    w2m = f32(conv2_w).reshape(C, C * 4)
    b1m = f32(conv1_b).reshape(1, C)
    b2m = f32(conv2_b).reshape(1, C)
    lwm = f32(lin_w)                               # [2, 12]
    lbm = f32(lin_b).reshape(1, 2)
    iden = np.eye(128, dtype=np.float32)
    # question boundary columns, gap rows zeroed to match the stacks
    qbnd = np.zeros((94, 2), dtype=np.float32)
    for t, base in enumerate((0, 32, 64)):
        qbnd[base:base + C, 0] = q[t]              # front: q(0), q(1), q(2)
        qbnd[base:base + C, 1] = q[Q - 3 + t]      # back
    # MLP weights: rows = [0 (simA); w_sim2; w_sim3; w_sim1; gaf(9); bias]
    wallm = np.zeros((14, 2), dtype=np.float32)
    wallm[1] = lwm[:, 1]
    wallm[2] = lwm[:, 2]
    wallm[3] = lwm[:, 0]
    wallm[4:13] = lwm[:, 3:12].T
    wallm[13] = lbm[0]

    in_maps = []
    for k in range(NCORES):
        sl = slice(k * NS, (k + 1) * NS)
        aux = np.zeros((16, W + 1), dtype=np.float32)
        aux[0, 0:NS] = float(S)       # pool output length, sentences (512)
        aux[0, QC] = float(Q)         # pool output length, question (128)
        aux[1, 0:NS] = S + 3.0        # conv output length, sentences (515)
        aux[1, QC] = Q + 3.0          # conv output length, question (131)
        aux[2, 0] = 1.0 if k == 0 else 0.0      # first-sentence mask
        aux[3:12, 0:NS] = gaf[sl].T             # gaf features, transposed
        aux[12, 0:NS] = 1.0                     # ones row (bias feature)
        aux[13, 0:4] = [1.0, 0.0, 0.0, -1.0]    # sdif selector
        # G: picks [|Sq|^2, |qg1|^2, |qg2|^2, |qg2|^2] from the nrm q-col
        G = np.zeros((4, 4), dtype=np.float32)
        G[0, 0] = G[1, 1] = G[2, 2] = G[2, 3] = 1.0
        aux[14, 0:16] = G.reshape(16)
        aux[15, 0:2] = lwm[:, 0]                # sim1 weights (lcorr)
        in_maps.append({
            "xs": np.ascontiguousarray(x[sl]),
            "qe": q,
            "w1": w1m, "b1": b1m, "w2": w2m, "b2": b2m,
            "aux": aux,
            "qbnd": qbnd,
            "slab": (2.0 * np.ascontiguousarray(lab[sl]).reshape(NS, 1)
                     - 1.0).astype(np.float32),
            "wall": wallm,
            "iden": iden,
        })
    return in_maps


def kernel(question_embeds, sents_embeds, sents_gaf, sents_labels,
           conv1_w, conv1_b, conv2_w, conv2_b, lin_w, lin_b):
    in_maps = _make_in_maps(question_embeds, sents_embeds, sents_gaf,
                            sents_labels, conv1_w, conv1_b, conv2_w, conv2_b,
                            lin_w, lin_b)
    nc = _get_nc()
    trace = bool(int(os.environ.get("BASSK_TRACE", "0")))
    res = run_bass_kernel_spmd(nc, in_maps, core_ids=list(range(NCORES)),
                               trace=trace)
    if trace and res.exec_time_ns is not None:
        print(f"HW exec time: {res.exec_time_ns} ns")
        if res.instructions_and_trace is not None:
            print("trace:", res.instructions_and_trace[1])

    emit = np.concatenate([res.results[k]["out"][0, 0:NS]
                           for k in range(NCORES)]).astype(np.float32)
    cost = np.float32(sum(float(res.results[k]["out"][0, NS])
                          for k in range(NCORES)))
    return np.asarray(cost), emit


if __name__ == "__main__":
    _build()
    print("build + compile OK")


# revision 17
# speedup vs baseline: 1.1723x; 1.1723x over previous
"""ABCNN3 distributed Bass kernel for 8 TRN2 NeuronCores.

Key algebraic insight: the reference only consumes *global average pools* of
the conv outputs, and conv + avg-pool are linear.  So for each sentence the
whole conv stack collapses to functions of (a) the per-channel sum over time
S[c] = sum_t x[c,t] and (b) the first/last 3 time-columns (pool edge
corrections).  The kernel is therefore a memory-bound row-sum over the big
[1024, 512, 30] tensor followed by tiny 30x30 matmuls, cosine sims, and a
2-class MLP.  Cosine similarity is scale-invariant, so all pooled vectors are
kept unnormalized; bias terms are folded into the matmuls via constant rows
(515/131 and 512/128 length constants).  The 2-class log-softmax reduces to
sigmoid/log-sigmoid of the logit difference.

Sharding: data-parallel over the sentence axis N (128 sentences/core),
question + parameters replicated.  Each core emits its 128 softmax outputs
plus its partial NLL sum; the host gather concatenates the former and adds
the latter (the unshard step for a mean-reduced scalar).
"""

import os
import sys

import numpy as np

for _p in ("/opt/trn_rl_repo",):
    if _p not in sys.path and os.path.isdir(_p):
        sys.path.insert(0, _p)

import concourse.bacc as bacc
import concourse.mybir as mybir
import concourse.tile as tile
from concourse.bass_utils import run_bass_kernel_spmd

F32 = mybir.dt.float32
U32 = mybir.dt.uint32
AX = mybir.AxisListType
ALU = mybir.AluOpType
ACTF = mybir.ActivationFunctionType

NCORES = 8
N, S, C, Q = 1024, 512, 30, 128
NS = N // NCORES          # sentences per core
CH = 8                    # DMA/reduce chunks along t
TCH = S // CH             # t per chunk
CW = TCH * C              # floats per chunk per sentence
W = NS + 1                # matmul width: 128 sentence cols + 1 question col
QC = NS                   # question column index

_CACHE = {}


def _build():
    nc = bacc.Bacc("TRN2", target_bir_lowering=False, debug=False,
                   enable_asserts=False, num_devices=NCORES)

    xs = nc.dram_tensor("xs", [NS, S * C], F32, kind="ExternalInput").ap()
    qe = nc.dram_tensor("qe", [Q, C], F32, kind="ExternalInput").ap()
    w1 = nc.dram_tensor("w1", [C, C * 4], F32, kind="ExternalInput").ap()
    b1 = nc.dram_tensor("b1", [1, C], F32, kind="ExternalInput").ap()
    w2 = nc.dram_tensor("w2", [C, C * 4], F32, kind="ExternalInput").ap()
    b2 = nc.dram_tensor("b2", [1, C], F32, kind="ExternalInput").ap()
    aux = nc.dram_tensor("aux", [16, W + 1], F32, kind="ExternalInput").ap()
    qbnd = nc.dram_tensor("qbnd", [94, 2], F32, kind="ExternalInput").ap()
    slab = nc.dram_tensor("slab", [NS, 1], F32, kind="ExternalInput").ap()
    wall = nc.dram_tensor("wall", [14, 2], F32, kind="ExternalInput").ap()
    iden = nc.dram_tensor("iden", [128, 128], F32, kind="ExternalInput").ap()
    out = nc.dram_tensor("out", [1, W], F32, kind="ExternalOutput").ap()

    with tile.TileContext(nc) as tc:
        with (
            tc.tile_pool(name="sb", bufs=1) as sb,
            tc.tile_pool(name="ps", bufs=7, space="PSUM") as ps,
        ):
            # ---------------- persistent SBUF tiles ----------------
            xt = [sb.tile([NS, CW], F32, tag=f"x{i}", name=f"x{i}")
                  for i in range(CH)]
            acc = sb.tile([NS, CW], F32, tag="acc", name="acc")
            q_sb = sb.tile([Q, C], F32, tag="q_sb", name="q_sb")
            w1_sb = sb.tile([C, C * 4], F32, tag="w1_sb", name="w1_sb")
            w2_sb = sb.tile([C, C * 4], F32, tag="w2_sb", name="w2_sb")
            slab_sb = sb.tile([NS, 1], F32, tag="slab_sb", name="slab_sb")
            iden_sb = sb.tile([128, 128], F32, tag="iden_sb", name="iden_sb")
            mask_sb = sb.tile([1, 1], F32, tag="mask_sb", name="mask_sb")
            ones_sb = sb.tile([128, 1], F32, tag="ones_sb", name="ones_sb")

            # stacked rhs/lhs for the collapsed conv matmuls.
            # SF rows: 0-29 S^T | 30 len2 | 31 len1 | 32-61 x(0) | 62-63 0 |
            #          64-93 x(1) | 94-95 0 | 96-125 x(2)
            SF = sb.tile([126, W], F32, tag="SF", name="SF")
            # Bstk rows: 0-29 x(L-3) | 30-31 0 | 32-61 x(L-2) | 62-63 0 |
            #            64-93 x(L-1)
            Bstk = sb.tile([94, W], F32, tag="Bstk", name="Bstk")
            rhs_Z = sb.tile([32, W], F32, tag="rhs_Z", name="rhs_Z")
            lhsT_SF = sb.tile([126, C], F32, tag="lhsT_SF", name="lhsT_SF")
            lhsT_B = sb.tile([94, C], F32, tag="lhsT_B", name="lhsT_B")
            lhsT1x = sb.tile([32, C], F32, tag="lhsT1x", name="lhsT1x")
            lhsT2x = sb.tile([32, C], F32, tag="lhsT2x", name="lhsT2x")
            # post-conv stack: 0-29 S | 32-61 sg1 | 64-93 sg2
            stk = sb.tile([94, W], F32, tag="stk", name="stk")
            sqk = sb.tile([94, W], F32, tag="sqk", name="sqk")
            lhsT_q = sb.tile([94, 4], F32, tag="lhsT_q", name="lhsT_q")
            mask4 = sb.tile([94, 4], F32, tag="mask4", name="mask4")
            sdif = sb.tile([4, 1], F32, tag="sdif", name="sdif")
            G_sb = sb.tile([4, 4], F32, tag="G_sb", name="G_sb")
            lwr0 = sb.tile([1, 2], F32, tag="lwr0", name="lwr0")
            wall_sb = sb.tile([14, 2], F32, tag="wall_sb", name="wall_sb")
            # rows 0-3 get the sims (DVE), rows 4-13 gaf+ones (DMA)
            mlp_all = sb.tile([14, W], F32, tag="mlp_all", name="mlp_all")
            warm = sb.tile([1, 1], F32, tag="warm", name="warm")
            outst = sb.tile([1, W], F32, tag="outst", name="outst")
            magic = sb.tile([4, W], U32, tag="magic", name="magic")

            # ---------------- zero/const fills ----------------
            for t in (SF, Bstk, rhs_Z, lhsT_SF, lhsT_B, lhsT1x, lhsT2x,
                      stk, lhsT_q, mask4):
                nc.gpsimd.memset(t[:], 0.0)
            nc.gpsimd.memset(ones_sb[:], 1.0)
            nc.gpsimd.memset(magic[:], 0x5f3759df)

            # ---------------- input DMAs ----------------
            # big x chunks: 4 on each HWDGE ring; all small DMAs go on the
            # sync ring so the scalar engine stays free for compute
            nc.sync.dma_start(iden_sb[:], iden)
            nc.scalar.dma_start(w1_sb[:], w1)
            nc.scalar.dma_start(w2_sb[:], w2)
            nc.sync.dma_start(q_sb[:], qe)
            for i in range(CH):
                eng = nc.sync if i % 2 == 0 else nc.scalar
                eng.dma_start(xt[i][:], xs[:, i * CW:(i + 1) * CW])
            nc.sync.dma_start(slab_sb[:], slab)
            nc.sync.dma_start(mask_sb[:], aux[2:3, 0:1])
            nc.sync.dma_start(SF[30:32, 0:W], aux[0:2, 0:W])   # len2, len1
            nc.sync.dma_start(rhs_Z[31:32, 0:W], aux[1:2, 0:W])  # len1
            nc.sync.dma_start(lhsT_SF[30:31, 0:C], b1)
            nc.sync.dma_start(lhsT1x[31:32, 0:C], b1)
            nc.sync.dma_start(lhsT2x[31:32, 0:C], b2)
            # question boundary columns (gap rows pre-zeroed on host)
            nc.sync.dma_start(SF[32:126, QC:QC + 1], qbnd[:, 0:1])
            nc.sync.dma_start(Bstk[0:94, QC:QC + 1], qbnd[:, 1:2])
            nc.sync.dma_start(mlp_all[4:14, 0:NS], aux[3:13, 0:NS])
            nc.sync.dma_start(sdif[:], aux[13:14, 0:4].rearrange("r c -> c r"))
            nc.sync.dma_start(
                G_sb[:], aux[14:15, 0:16].rearrange("a (p j) -> (a p) j", p=4))
            nc.sync.dma_start(lwr0[:], aux[15:16, 0:2])
            nc.sync.dma_start(wall_sb[:], wall)

            # sigmoid table warmup: loads the set early so the tail sigmoid
            # runs without an ACT_TABLE_LOAD (copies are in every set)
            nc.scalar.activation(warm[:], ones_sb[0:1, 0:1], ACTF.Sigmoid)

            # mask4: block indicator columns; col3 reuses the S block
            for r, base in ((0, 0), (1, 32), (2, 64), (3, 0)):
                nc.vector.tensor_copy(mask4[base:base + C, r:r + 1],
                                      ones_sb[base:base + C, 0:1])

            # ---------------- weight prep (all tiny, overlaps DMA) -------
            def wk1(k):
                return w1_sb[:, k:C * 4:4]

            def wk2(k):
                return w2_sb[:, k:C * 4:4]

            wk = wk1
            W1s = sb.tile([C, C], F32, tag="W1s", name="W1s")
            W2s = sb.tile([C, C], F32, tag="W2s", name="W2s")
            nc.vector.tensor_reduce(
                W1s[:], w1_sb[:].rearrange("p (i k) -> p i k", k=4),
                axis=AX.X, op=ALU.add)
            nc.vector.tensor_reduce(
                W2s[:], w2_sb[:].rearrange("p (i k) -> p i k", k=4),
                axis=AX.X, op=ALU.add)

            t1 = sb.tile([C, C], F32, tag="t1", name="t1")
            t2 = sb.tile([C, C], F32, tag="t2", name="t2")
            NF0 = sb.tile([C, C], F32, tag="NF0", name="NF0")
            NF1 = sb.tile([C, C], F32, tag="NF1", name="NF1")
            NF2 = sb.tile([C, C], F32, tag="NF2", name="NF2")
            NB0 = sb.tile([C, C], F32, tag="NB0", name="NB0")
            NB1 = sb.tile([C, C], F32, tag="NB1", name="NB1")
            NB2 = sb.tile([C, C], F32, tag="NB2", name="NB2")

            # front coeffs (negated): NF0 -> x[0], NF1 -> x[1], NF2 -> x[2]
            nc.vector.tensor_scalar(t1[:], wk(2), 2.0, None, ALU.mult)
            nc.vector.tensor_add(t1[:], t1[:], wk(1))
            nc.vector.tensor_scalar(t2[:], wk(3), 3.0, None, ALU.mult)
            nc.vector.tensor_add(t1[:], t1[:], t2[:])
            nc.vector.tensor_scalar(NF0[:], t1[:], -0.25, None, ALU.mult)
            nc.vector.tensor_scalar(t1[:], wk(3), 2.0, None, ALU.mult)
            nc.vector.tensor_add(t1[:], t1[:], wk(2))
            nc.vector.tensor_scalar(NF1[:], t1[:], -0.25, None, ALU.mult)
            nc.vector.tensor_scalar(NF2[:], wk(3), -0.25, None, ALU.mult)
            # back coeffs (negated): NB0 -> x[L-3], NB1 -> x[L-2], NB2 -> x[L-1]
            nc.vector.tensor_scalar(NB0[:], wk(0), -0.25, None, ALU.mult)
            nc.vector.tensor_scalar(t1[:], wk(0), 2.0, None, ALU.mult)
            nc.vector.tensor_add(t1[:], t1[:], wk(1))
            nc.vector.tensor_scalar(NB1[:], t1[:], -0.25, None, ALU.mult)
            nc.vector.tensor_scalar(t1[:], wk(0), 3.0, None, ALU.mult)
            nc.vector.tensor_scalar(t2[:], wk(1), 2.0, None, ALU.mult)
            nc.vector.tensor_add(t1[:], t1[:], t2[:])
            nc.vector.tensor_add(t1[:], t1[:], wk(2))
            nc.vector.tensor_scalar(NB2[:], t1[:], -0.25, None, ALU.mult)

            id30 = iden_sb[0:C, 0:C]
            # small-weight transposes via regular matmuls (out = src^T) so
            # they can land at quadrant bases of one psum tile
            psL = ps.tile([126, C], F32, tag="ps", name="psL")
            for src, base in ((W1s, 0), (NF0, 32), (NF1, 64), (NF2, 96)):
                nc.tensor.matmul(psL[base:base + C, 0:C], src[:], id30,
                                 tile_position=(0, base))
            for base in (0, 32, 64, 96):
                nc.scalar.copy(lhsT_SF[base:base + C, 0:C],
                               psL[base:base + C, 0:C])
            nc.scalar.copy(lhsT1x[0:C, 0:C], psL[0:C, 0:C])
            psLB = ps.tile([94, C], F32, tag="ps", name="psLB")
            for src, base in ((NB0, 0), (NB1, 32), (NB2, 64)):
                nc.tensor.matmul(psLB[base:base + C, 0:C], src[:], id30,
                                 tile_position=(0, base))
            for base in (0, 32, 64):
                nc.scalar.copy(lhsT_B[base:base + C, 0:C],
                               psLB[base:base + C, 0:C])
            psW2 = ps.tile([C, C], F32, tag="ps", name="psW2")
            nc.tensor.matmul(psW2[:], W2s[:], id30)
            nc.scalar.copy(lhsT2x[0:C, 0:C], psW2[:])

            # x boundary transposes (need only chunks 0 and 7)
            psSF = ps.tile([126, W], F32, tag="ps", name="psSF")
            for t, base in enumerate((32, 64, 96)):
                nc.tensor.matmul(psSF[base:base + C, 0:NS],
                                 xt[0][:, t * C:(t + 1) * C],
                                 iden_sb[:, :], tile_position=(0, base))
                nc.scalar.copy(SF[base:base + C, 0:NS],
                               psSF[base:base + C, 0:NS])
            psB = ps.tile([94, NS], F32, tag="ps", name="psB")
            for t, base in enumerate((0, 32, 64)):
                off = CW - 90 + t * C
                nc.tensor.matmul(psB[base:base + C, 0:NS],
                                 xt[CH - 1][:, off:off + C],
                                 iden_sb[:, :], tile_position=(0, base))
                nc.scalar.copy(Bstk[base:base + C, 0:NS],
                               psB[base:base + C, 0:NS])

            # PE clock warm-up: keep TensorE busy through the DVE reduce
            # phase so the tail matmuls run at the boosted clock.
            warm_ps = ps.tile([C, NS], F32, tag="warm_ps", name="warm_ps",
                              bufs=1)
            for _wi in range(44):
                nc.tensor.matmul(warm_ps[:], iden_sb[0:C, 0:C],
                                 iden_sb[0:C, 0:NS])

            # ---------------- big reduction: S[n,c] = sum_t x ------------
            nc.vector.tensor_add(acc[:], xt[0][:], xt[1][:])
            for i in range(2, CH):
                nc.vector.tensor_add(acc[:], acc[:], xt[i][:])
            w_ = CW
            while w_ > C:
                h = w_ // 2
                nc.vector.tensor_add(acc[:, 0:h], acc[:, 0:h], acc[:, h:w_])
                w_ = h
            # S = acc[:, 0:30]  (per-sentence channel sums)

            # S^T into SF rows 0-29 (+ question column Sq)
            nc.tensor.transpose(psSF[0:C, 0:NS], acc[:, 0:C], iden_sb[:, :])
            nc.tensor.matmul(psSF[0:C, QC:QC + 1], q_sb[:], ones_sb[:, :])
            nc.scalar.copy(SF[0:C, 0:W], psSF[0:C, 0:W])

            # ---------------- collapsed conv matmuls ----------------
            # Z (window-pooled conv1 channel sums), sentences + question
            psZ = ps.tile([C, W], F32, tag="ps", name="psZ")
            nc.tensor.matmul(psZ[:], lhsT_SF[0:126, 0:C], SF[0:126, 0:W],
                             start=True, stop=False)
            nc.tensor.matmul(psZ[:], lhsT_B[0:94, 0:C], Bstk[0:94, 0:W],
                             start=False, stop=True)
            nc.scalar.copy(rhs_Z[0:C, 0:W], psZ[:])

            # post-conv stack in one psum tile: sg1@32, sg2@64
            psStk = ps.tile([94, W], F32, tag="ps", name="psStk")
            nc.tensor.matmul(psStk[32:62, 0:W], lhsT1x[0:32, 0:C],
                             SF[0:32, 0:W], tile_position=(0, 32))
            nc.tensor.matmul(psStk[64:94, 0:W], lhsT2x[0:32, 0:C],
                             rhs_Z[0:32, 0:W], tile_position=(0, 64))
            # qg2 again at base 0 (for the qg2 . S similarity column)
            psQ2 = ps.tile([C, 1], F32, tag="ps", name="psQ2")
            nc.tensor.matmul(psQ2[:], lhsT2x[0:32, 0:C],
                             rhs_Z[0:32, QC:QC + 1])
            nc.vector.tensor_copy(stk[0:C, 0:W], SF[0:C, 0:W])
            nc.scalar.copy(stk[32:62, 0:W], psStk[32:62, 0:W])
            nc.scalar.copy(stk[64:94, 0:W], psStk[64:94, 0:W])

            # squares for the norms; q-vector columns for the dots
            nc.vector.tensor_mul(sqk[:], stk[:], stk[:])
            for r, base in ((0, 0), (1, 32), (2, 64)):
                nc.vector.tensor_copy(lhsT_q[base:base + C, r:r + 1],
                                      stk[base:base + C, QC:QC + 1])
            nc.vector.tensor_copy(lhsT_q[0:C, 3:4], psQ2[:])

            # dots4 rows: 0 = Sq.S (simA), 1 = qg1.sg1, 2 = qg2.sg2,
            #             3 = qg2.S (simB);  nrm4 rows: |S|,|sg1|,|sg2|,|S|^2
            dots4 = ps.tile([4, W], F32, tag="ps", name="dots4")
            nc.tensor.matmul(dots4[:], lhsT_q[0:94, 0:4], stk[0:94, 0:W])
            nrm4 = ps.tile([4, W], F32, tag="ps", name="nrm4")
            nc.tensor.matmul(nrm4[:], mask4[0:94, 0:4], sqk[0:94, 0:W])

            # q-side squared norms: [|Sq|^2,|qg1|^2,|qg2|^2,|qg2|^2] via G
            qncol = sb.tile([4, 1], F32, tag="qncol", name="qncol")
            nc.scalar.copy(qncol[:], nrm4[0:4, QC:QC + 1])
            psQn = ps.tile([4, 1], F32, tag="ps", name="psQn")
            nc.tensor.matmul(psQn[:], G_sb[:], qncol[:])
            qn = sb.tile([4, 1], F32, tag="qn", name="qn")
            nc.scalar.copy(qn[:], psQn[:])
            ppack = sb.tile([4, W], F32, tag="ppack", name="ppack")
            nc.vector.tensor_scalar(ppack[:], nrm4[0:4, 0:W], qn[:],
                                    None, ALU.mult)

            # rsqrt(ppack) on DVE: magic-number seed + 2 Newton steps
            ish = sb.tile([4, W], U32, tag="ish", name="ish")
            nc.vector.tensor_scalar(ish[:], ppack[:].bitcast(U32), 1,
                                    None, ALU.logical_shift_right)
            y0 = sb.tile([4, W], F32, tag="y0", name="y0")
            nc.vector.tensor_sub(y0[:].bitcast(U32), magic[:], ish[:])
            na = sb.tile([4, W], F32, tag="na", name="na")
            nb = sb.tile([4, W], F32, tag="nb", name="nb")
            nc.vector.tensor_mul(na[:], y0[:], y0[:])
            nc.vector.tensor_mul(nb[:], na[:], ppack[:])
            nc.vector.tensor_scalar(nb[:], nb[:], -0.5, 1.5, ALU.mult,
                                    ALU.add)
            rs = sb.tile([4, W], F32, tag="rs", name="rs")
            nc.vector.tensor_mul(rs[:], y0[:], nb[:])
            # second Newton step for accuracy (cheap, [4,W] only)
            nc.vector.tensor_mul(na[:], rs[:], rs[:])
            nc.vector.tensor_mul(nb[:], na[:], ppack[:])
            nc.vector.tensor_scalar(nb[:], nb[:], -0.5, 1.5, ALU.mult,
                                    ALU.add)
            nc.vector.tensor_mul(rs[:], rs[:], nb[:])

            # sims straight into the MLP lhsT rows 0-3
            nc.vector.tensor_mul(mlp_all[0:4, 0:W], dots4[0:4, 0:W], rs[:])

            # sim1 = simB, except sentence 0 of core 0 -> simA.
            dif_ps = ps.tile([1, 1], F32, tag="ps", name="dif_ps")
            nc.tensor.matmul(dif_ps[:], sdif[0:4, 0:1], mlp_all[0:4, 0:1])
            md = sb.tile([1, 1], F32, tag="md", name="md")
            nc.vector.tensor_mul(md[:], dif_ps[:], mask_sb[:])
            lcorr = sb.tile([1, 2], F32, tag="lcorr", name="lcorr")
            nc.vector.tensor_scalar(lcorr[:], lwr0[:], md[:], None, ALU.mult)

            # ---------------- MLP + 2-class softmax + outputs ------------
            logits = ps.tile([NS, 2], F32, tag="ps", name="logits")
            nc.tensor.matmul(logits[:], mlp_all[0:14, 0:NS], wall_sb[:])
            nc.vector.tensor_add(logits[0:1, 0:2], logits[0:1, 0:2],
                                 lcorr[:])
            lg_sb = sb.tile([NS, 2], F32, tag="lg_sb", name="lg_sb")
            nc.vector.tensor_copy(lg_sb[:], logits[:])
            dz = sb.tile([NS, 1], F32, tag="dz", name="dz")
            nc.vector.tensor_sub(dz[:], lg_sb[:, 1:2], lg_sb[:, 0:1])
            sdz = sb.tile([NS, 1], F32, tag="sdz", name="sdz")
            nc.vector.tensor_mul(sdz[:], dz[:], slab_sb[:])
            # emit = sigmoid(dz); picked log-prob = ln(sigmoid(sdz))
            em = sb.tile([NS, 1], F32, tag="em", name="em")
            nc.scalar.activation(em[:], dz[:], ACTF.Sigmoid)
            psel = sb.tile([NS, 1], F32, tag="psel", name="psel")
            nc.scalar.activation(psel[:], sdz[:], ACTF.Sigmoid)
            pick = sb.tile([NS, 1], F32, tag="pick", name="pick")
            nc.scalar.activation(pick[:], psel[:], ACTF.Ln)
            costp = ps.tile([1, 1], F32, tag="ps", name="costp")
            nc.tensor.matmul(costp[:], ones_sb[:, :], pick[:])
            # partial cost (already scaled); host sums the 8 partials
            nc.vector.tensor_scalar(outst[0:1, NS:NS + 1], costp[:],
                                    -1.0 / N, None, ALU.mult)
            psE = ps.tile([1, NS], F32, tag="ps", name="psE")
            nc.tensor.transpose(psE[:], em[:], iden_sb[:, :])
            nc.vector.tensor_copy(outst[0:1, 0:NS], psE[:])
            nc.sync.dma_start(out[0:1, 0:W], outst[:])

    nc.compile()
    return nc


def _get_nc():
    if "nc" not in _CACHE:
        _CACHE["nc"] = _build()
    return _CACHE["nc"]


def _make_in_maps(question_embeds, sents_embeds, sents_gaf, sents_labels,
                  conv1_w, conv1_b, conv2_w, conv2_b, lin_w, lin_b):
    f32 = lambda a: np.ascontiguousarray(np.asarray(a), dtype=np.float32)
    q = f32(question_embeds)                       # [128, 30]
    x = f32(sents_embeds).reshape(N, S * C)        # [1024, 15360]
    gaf = f32(sents_gaf)                           # [1024, 9]
    lab = f32(np.asarray(sents_labels))            # [1024]
    w1m = f32(conv1_w).reshape(C, C * 4)
    w2m = f32(conv2_w).reshape(C, C * 4)
    b1m = f32(conv1_b).reshape(1, C)
    b2m = f32(conv2_b).reshape(1, C)
    lwm = f32(lin_w)                               # [2, 12]
    lbm = f32(lin_b).reshape(1, 2)
    iden = np.eye(128, dtype=np.float32)
    # question boundary columns, gap rows zeroed to match the stacks
    qbnd = np.zeros((94, 2), dtype=np.float32)
    for t, base in enumerate((0, 32, 64)):
        qbnd[base:base + C, 0] = q[t]              # front: q(0), q(1), q(2)
        qbnd[base:base + C, 1] = q[Q - 3 + t]      # back
    # MLP weights: rows = [0 (simA); w_sim2; w_sim3; w_sim1; gaf(9); bias]
    wallm = np.zeros((14, 2), dtype=np.float32)
    wallm[1] = lwm[:, 1]
    wallm[2] = lwm[:, 2]
    wallm[3] = lwm[:, 0]
    wallm[4:13] = lwm[:, 3:12].T
    wallm[13] = lbm[0]

    in_maps = []
    for k in range(NCORES):
        sl = slice(k * NS, (k + 1) * NS)
        aux = np.zeros((16, W + 1), dtype=np.float32)
        aux[0, 0:NS] = float(S)       # pool output length, sentences (512)
        aux[0, QC] = float(Q)         # pool output length, question (128)
        aux[1, 0:NS] = S + 3.0        # conv output length, sentences (515)
        aux[1, QC] = Q + 3.0          # conv output length, question (131)
        aux[2, 0] = 1.0 if k == 0 else 0.0      # first-sentence mask
        aux[3:12, 0:NS] = gaf[sl].T             # gaf features, transposed
        aux[12, 0:NS] = 1.0                     # ones row (bias feature)
        aux[13, 0:4] = [1.0, 0.0, 0.0, -1.0]    # sdif selector
        # G: picks [|Sq|^2, |qg1|^2, |qg2|^2, |qg2|^2] from the nrm q-col
        G = np.zeros((4, 4), dtype=np.float32)
        G[0, 0] = G[1, 1] = G[2, 2] = G[2, 3] = 1.0
        aux[14, 0:16] = G.reshape(16)
        aux[15, 0:2] = lwm[:, 0]                # sim1 weights (lcorr)
        in_maps.append({
            "xs": np.ascontiguousarray(x[sl]),
            "qe": q,
            "w1": w1m, "b1": b1m, "w2": w2m, "b2": b2m,
            "aux": aux,
            "qbnd": qbnd,
            "slab": (2.0 * np.ascontiguousarray(lab[sl]).reshape(NS, 1)
                     - 1.0).astype(np.float32),
            "wall": wallm,
            "iden": iden,
        })
    return in_maps


def kernel(question_embeds, sents_embeds, sents_gaf, sents_labels,
           conv1_w, conv1_b, conv2_w, conv2_b, lin_w, lin_b):
    in_maps = _make_in_maps(question_embeds, sents_embeds, sents_gaf,
                            sents_labels, conv1_w, conv1_b, conv2_w, conv2_b,
                            lin_w, lin_b)
    nc = _get_nc()
    trace = bool(int(os.environ.get("BASSK_TRACE", "0")))
    res = run_bass_kernel_spmd(nc, in_maps, core_ids=list(range(NCORES)),
                               trace=trace)
    if trace and res.exec_time_ns is not None:
        print(f"HW exec time: {res.exec_time_ns} ns")
        if res.instructions_and_trace is not None:
            print("trace:", res.instructions_and_trace[1])

    emit = np.concatenate([res.results[k]["out"][0, 0:NS]
                           for k in range(NCORES)]).astype(np.float32)
    cost = np.float32(sum(float(res.results[k]["out"][0, NS])
                          for k in range(NCORES)))
    return np.asarray(cost), emit


if __name__ == "__main__":
    _build()
    print("build + compile OK")


# revision 18
# speedup vs baseline: 1.2365x; 1.0548x over previous
"""ABCNN3 distributed Bass kernel for 8 TRN2 NeuronCores.

Key algebraic insight: the reference only consumes *global average pools* of
the conv outputs, and conv + avg-pool are linear.  So for each sentence the
whole conv stack collapses to functions of (a) the per-channel sum over time
S[c] = sum_t x[c,t] and (b) the first/last 3 time-columns (pool edge
corrections).  The kernel is therefore a memory-bound row-sum over the big
[1024, 512, 30] tensor followed by tiny 30x30 matmuls, cosine sims, and a
2-class MLP.  Cosine similarity is scale-invariant, so all pooled vectors are
kept unnormalized; bias terms are folded into the matmuls via constant rows
(515/131 and 512/128 length constants).  The 2-class log-softmax reduces to
sigmoid/log-sigmoid of the logit difference.

Sharding: data-parallel over the sentence axis N (128 sentences/core),
question + parameters replicated.  Each core emits its 128 softmax outputs
plus its partial NLL sum; the host gather concatenates the former and adds
the latter (the unshard step for a mean-reduced scalar).
"""

import os
import sys

import numpy as np

for _p in ("/opt/trn_rl_repo",):
    if _p not in sys.path and os.path.isdir(_p):
        sys.path.insert(0, _p)

import concourse.bacc as bacc
import concourse.mybir as mybir
import concourse.tile as tile
from concourse.bass_utils import run_bass_kernel_spmd

F32 = mybir.dt.float32
U32 = mybir.dt.uint32
AX = mybir.AxisListType
ALU = mybir.AluOpType
ACTF = mybir.ActivationFunctionType

NCORES = 8
N, S, C, Q = 1024, 512, 30, 128
NS = N // NCORES          # sentences per core
CH = 16                   # DMA/reduce chunks along t
TCH = S // CH             # t per chunk
CW = TCH * C              # floats per chunk per sentence
W = NS + 1                # matmul width: 128 sentence cols + 1 question col
QC = NS                   # question column index

_CACHE = {}


def _build():
    nc = bacc.Bacc("TRN2", target_bir_lowering=False, debug=False,
                   enable_asserts=False, num_devices=NCORES)

    xs = nc.dram_tensor("xs", [NS, S * C], F32, kind="ExternalInput").ap()
    qe = nc.dram_tensor("qe", [Q, C], F32, kind="ExternalInput").ap()
    w1 = nc.dram_tensor("w1", [C, C * 4], F32, kind="ExternalInput").ap()
    b1 = nc.dram_tensor("b1", [1, C], F32, kind="ExternalInput").ap()
    w2 = nc.dram_tensor("w2", [C, C * 4], F32, kind="ExternalInput").ap()
    b2 = nc.dram_tensor("b2", [1, C], F32, kind="ExternalInput").ap()
    aux = nc.dram_tensor("aux", [16, W + 1], F32, kind="ExternalInput").ap()
    qbnd = nc.dram_tensor("qbnd", [94, 2], F32, kind="ExternalInput").ap()
    slab = nc.dram_tensor("slab", [NS, 1], F32, kind="ExternalInput").ap()
    wall = nc.dram_tensor("wall", [14, 2], F32, kind="ExternalInput").ap()
    iden = nc.dram_tensor("iden", [128, 128], F32, kind="ExternalInput").ap()
    out = nc.dram_tensor("out", [1, W], F32, kind="ExternalOutput").ap()

    with tile.TileContext(nc) as tc:
        with (
            tc.tile_pool(name="sb", bufs=1) as sb,
            tc.tile_pool(name="ps", bufs=7, space="PSUM") as ps,
        ):
            # ---------------- persistent SBUF tiles ----------------
            xt = [sb.tile([NS, CW], F32, tag=f"x{i}", name=f"x{i}")
                  for i in range(CH)]
            acc = sb.tile([NS, CW], F32, tag="acc", name="acc")
            q_sb = sb.tile([Q, C], F32, tag="q_sb", name="q_sb")
            w1_sb = sb.tile([C, C * 4], F32, tag="w1_sb", name="w1_sb")
            w2_sb = sb.tile([C, C * 4], F32, tag="w2_sb", name="w2_sb")
            slab_sb = sb.tile([NS, 1], F32, tag="slab_sb", name="slab_sb")
            iden_sb = sb.tile([128, 128], F32, tag="iden_sb", name="iden_sb")
            mask_sb = sb.tile([1, 1], F32, tag="mask_sb", name="mask_sb")
            ones_sb = sb.tile([128, 1], F32, tag="ones_sb", name="ones_sb")

            # stacked rhs/lhs for the collapsed conv matmuls.
            # SF rows: 0-29 S^T | 30 len2 | 31 len1 | 32-61 x(0) | 62-63 0 |
            #          64-93 x(1) | 94-95 0 | 96-125 x(2)
            SF = sb.tile([126, W], F32, tag="SF", name="SF")
            # Bstk rows: 0-29 x(L-3) | 30-31 0 | 32-61 x(L-2) | 62-63 0 |
            #            64-93 x(L-1)
            Bstk = sb.tile([94, W], F32, tag="Bstk", name="Bstk")
            rhs_Z = sb.tile([32, W], F32, tag="rhs_Z", name="rhs_Z")
            lhsT_SF = sb.tile([126, C], F32, tag="lhsT_SF", name="lhsT_SF")
            lhsT_B = sb.tile([94, C], F32, tag="lhsT_B", name="lhsT_B")
            lhsT1x = sb.tile([32, C], F32, tag="lhsT1x", name="lhsT1x")
            lhsT2x = sb.tile([32, C], F32, tag="lhsT2x", name="lhsT2x")
            # post-conv stack: 0-29 S | 32-61 sg1 | 64-93 sg2
            stk = sb.tile([94, W], F32, tag="stk", name="stk")
            sqk = sb.tile([94, W], F32, tag="sqk", name="sqk")
            lhsT_q = sb.tile([94, 4], F32, tag="lhsT_q", name="lhsT_q")
            mask4 = sb.tile([94, 4], F32, tag="mask4", name="mask4")
            sdif = sb.tile([4, 1], F32, tag="sdif", name="sdif")
            G_sb = sb.tile([4, 4], F32, tag="G_sb", name="G_sb")
            lwr0 = sb.tile([1, 2], F32, tag="lwr0", name="lwr0")
            wall_sb = sb.tile([14, 2], F32, tag="wall_sb", name="wall_sb")
            # rows 0-3 get the sims (DVE), rows 4-13 gaf+ones (DMA)
            mlp_all = sb.tile([14, W], F32, tag="mlp_all", name="mlp_all")
            warm = sb.tile([1, 1], F32, tag="warm", name="warm")
            outst = sb.tile([1, W], F32, tag="outst", name="outst")
            magic = sb.tile([4, W], U32, tag="magic", name="magic")

            # ---------------- zero/const fills ----------------
            for t in (SF, Bstk, rhs_Z, lhsT_SF, lhsT_B, lhsT1x, lhsT2x,
                      stk, lhsT_q, mask4):
                nc.gpsimd.memset(t[:], 0.0)
            nc.gpsimd.memset(ones_sb[:], 1.0)
            nc.gpsimd.memset(magic[:], 0x5f3759df)

            # ---------------- input DMAs ----------------
            # big x chunks: 4 on each HWDGE ring; all small DMAs go on the
            # sync ring so the scalar engine stays free for compute
            nc.sync.dma_start(iden_sb[:], iden)
            nc.scalar.dma_start(w1_sb[:], w1)
            nc.scalar.dma_start(w2_sb[:], w2)
            for i in range(CH):
                eng = nc.sync if i % 2 == 0 else nc.scalar
                eng.dma_start(xt[i][:], xs[:, i * CW:(i + 1) * CW])
            nc.sync.dma_start(q_sb[:], qe)
            nc.sync.dma_start(slab_sb[:], slab)
            nc.sync.dma_start(mask_sb[:], aux[2:3, 0:1])
            nc.sync.dma_start(SF[30:32, 0:W], aux[0:2, 0:W])   # len2, len1
            nc.sync.dma_start(rhs_Z[31:32, 0:W], aux[1:2, 0:W])  # len1
            nc.sync.dma_start(lhsT_SF[30:31, 0:C], b1)
            nc.sync.dma_start(lhsT1x[31:32, 0:C], b1)
            nc.sync.dma_start(lhsT2x[31:32, 0:C], b2)
            # question boundary columns (gap rows pre-zeroed on host)
            nc.sync.dma_start(SF[32:126, QC:QC + 1], qbnd[:, 0:1])
            nc.sync.dma_start(Bstk[0:94, QC:QC + 1], qbnd[:, 1:2])
            nc.sync.dma_start(mlp_all[4:14, 0:NS], aux[3:13, 0:NS])
            nc.sync.dma_start(sdif[:], aux[13:14, 0:4].rearrange("r c -> c r"))
            nc.sync.dma_start(
                G_sb[:], aux[14:15, 0:16].rearrange("a (p j) -> (a p) j", p=4))
            nc.sync.dma_start(lwr0[:], aux[15:16, 0:2])
            nc.sync.dma_start(wall_sb[:], wall)

            # sigmoid table warmup: loads the set early so the tail sigmoid
            # runs without an ACT_TABLE_LOAD (copies are in every set)
            nc.scalar.activation(warm[:], ones_sb[0:1, 0:1], ACTF.Sigmoid)

            # mask4: block indicator columns; col3 reuses the S block
            for r, base in ((0, 0), (1, 32), (2, 64), (3, 0)):
                nc.vector.tensor_copy(mask4[base:base + C, r:r + 1],
                                      ones_sb[base:base + C, 0:1])

            # ---------------- weight prep (all tiny, overlaps DMA) -------
            def wk1(k):
                return w1_sb[:, k:C * 4:4]

            def wk2(k):
                return w2_sb[:, k:C * 4:4]

            wk = wk1
            W1s = sb.tile([C, C], F32, tag="W1s", name="W1s")
            W2s = sb.tile([C, C], F32, tag="W2s", name="W2s")
            nc.vector.tensor_reduce(
                W1s[:], w1_sb[:].rearrange("p (i k) -> p i k", k=4),
                axis=AX.X, op=ALU.add)
            nc.vector.tensor_reduce(
                W2s[:], w2_sb[:].rearrange("p (i k) -> p i k", k=4),
                axis=AX.X, op=ALU.add)

            t1 = sb.tile([C, C], F32, tag="t1", name="t1")
            t2 = sb.tile([C, C], F32, tag="t2", name="t2")
            NF0 = sb.tile([C, C], F32, tag="NF0", name="NF0")
            NF1 = sb.tile([C, C], F32, tag="NF1", name="NF1")
            NF2 = sb.tile([C, C], F32, tag="NF2", name="NF2")
            NB0 = sb.tile([C, C], F32, tag="NB0", name="NB0")
            NB1 = sb.tile([C, C], F32, tag="NB1", name="NB1")
            NB2 = sb.tile([C, C], F32, tag="NB2", name="NB2")

            # front coeffs (negated): NF0 -> x[0], NF1 -> x[1], NF2 -> x[2]
            nc.vector.tensor_scalar(t1[:], wk(2), 2.0, None, ALU.mult)
            nc.vector.tensor_add(t1[:], t1[:], wk(1))
            nc.vector.tensor_scalar(t2[:], wk(3), 3.0, None, ALU.mult)
            nc.vector.tensor_add(t1[:], t1[:], t2[:])
            nc.vector.tensor_scalar(NF0[:], t1[:], -0.25, None, ALU.mult)
            nc.vector.tensor_scalar(t1[:], wk(3), 2.0, None, ALU.mult)
            nc.vector.tensor_add(t1[:], t1[:], wk(2))
            nc.vector.tensor_scalar(NF1[:], t1[:], -0.25, None, ALU.mult)
            nc.vector.tensor_scalar(NF2[:], wk(3), -0.25, None, ALU.mult)
            # back coeffs (negated): NB0 -> x[L-3], NB1 -> x[L-2], NB2 -> x[L-1]
            nc.vector.tensor_scalar(NB0[:], wk(0), -0.25, None, ALU.mult)
            nc.vector.tensor_scalar(t1[:], wk(0), 2.0, None, ALU.mult)
            nc.vector.tensor_add(t1[:], t1[:], wk(1))
            nc.vector.tensor_scalar(NB1[:], t1[:], -0.25, None, ALU.mult)
            nc.vector.tensor_scalar(t1[:], wk(0), 3.0, None, ALU.mult)
            nc.vector.tensor_scalar(t2[:], wk(1), 2.0, None, ALU.mult)
            nc.vector.tensor_add(t1[:], t1[:], t2[:])
            nc.vector.tensor_add(t1[:], t1[:], wk(2))
            nc.vector.tensor_scalar(NB2[:], t1[:], -0.25, None, ALU.mult)

            id30 = iden_sb[0:C, 0:C]
            # small-weight transposes via regular matmuls (out = src^T) so
            # they can land at quadrant bases of one psum tile
            psL = ps.tile([126, C], F32, tag="ps", name="psL")
            for src, base in ((W1s, 0), (NF0, 32), (NF1, 64), (NF2, 96)):
                nc.tensor.matmul(psL[base:base + C, 0:C], src[:], id30,
                                 tile_position=(0, base))
            for base in (0, 32, 64, 96):
                nc.scalar.copy(lhsT_SF[base:base + C, 0:C],
                               psL[base:base + C, 0:C])
            nc.scalar.copy(lhsT1x[0:C, 0:C], psL[0:C, 0:C])
            psLB = ps.tile([94, C], F32, tag="ps", name="psLB")
            for src, base in ((NB0, 0), (NB1, 32), (NB2, 64)):
                nc.tensor.matmul(psLB[base:base + C, 0:C], src[:], id30,
                                 tile_position=(0, base))
            for base in (0, 32, 64):
                nc.scalar.copy(lhsT_B[base:base + C, 0:C],
                               psLB[base:base + C, 0:C])
            psW2 = ps.tile([C, C], F32, tag="ps", name="psW2")
            nc.tensor.matmul(psW2[:], W2s[:], id30)
            nc.scalar.copy(lhsT2x[0:C, 0:C], psW2[:])

            # x boundary transposes (need only chunks 0 and 7)
            psSF = ps.tile([126, W], F32, tag="ps", name="psSF")
            for t, base in enumerate((32, 64, 96)):
                nc.tensor.matmul(psSF[base:base + C, 0:NS],
                                 xt[0][:, t * C:(t + 1) * C],
                                 iden_sb[:, :], tile_position=(0, base))
                nc.scalar.copy(SF[base:base + C, 0:NS],
                               psSF[base:base + C, 0:NS])
            psB = ps.tile([94, NS], F32, tag="ps", name="psB")
            for t, base in enumerate((0, 32, 64)):
                off = CW - 90 + t * C
                nc.tensor.matmul(psB[base:base + C, 0:NS],
                                 xt[CH - 1][:, off:off + C],
                                 iden_sb[:, :], tile_position=(0, base))
                nc.scalar.copy(Bstk[base:base + C, 0:NS],
                               psB[base:base + C, 0:NS])

            # PE clock warm-up: keep TensorE busy through the DVE reduce
            # phase so the tail matmuls run at the boosted clock.
            warm_ps = ps.tile([C, NS], F32, tag="warm_ps", name="warm_ps",
                              bufs=1)
            for _wi in range(44):
                nc.tensor.matmul(warm_ps[:], iden_sb[0:C, 0:C],
                                 iden_sb[0:C, 0:NS])

            # ---------------- big reduction: S[n,c] = sum_t x ------------
            nc.vector.tensor_add(acc[:], xt[0][:], xt[1][:])
            for i in range(2, CH):
                nc.vector.tensor_add(acc[:], acc[:], xt[i][:])
            w_ = CW
            while w_ > C:
                h = w_ // 2
                nc.vector.tensor_add(acc[:, 0:h], acc[:, 0:h], acc[:, h:w_])
                w_ = h
            # S = acc[:, 0:30]  (per-sentence channel sums)

            # S^T into SF rows 0-29 (+ question column Sq)
            nc.tensor.transpose(psSF[0:C, 0:NS], acc[:, 0:C], iden_sb[:, :])
            nc.tensor.matmul(psSF[0:C, QC:QC + 1], q_sb[:], ones_sb[:, :])
            nc.scalar.copy(SF[0:C, 0:W], psSF[0:C, 0:W])

            # ---------------- collapsed conv matmuls ----------------
            # Z (window-pooled conv1 channel sums), sentences + question
            psZ = ps.tile([C, W], F32, tag="ps", name="psZ")
            nc.tensor.matmul(psZ[:], lhsT_SF[0:126, 0:C], SF[0:126, 0:W],
                             start=True, stop=False)
            nc.tensor.matmul(psZ[:], lhsT_B[0:94, 0:C], Bstk[0:94, 0:W],
                             start=False, stop=True)
            nc.scalar.copy(rhs_Z[0:C, 0:W], psZ[:])

            # post-conv stack in one psum tile: sg1@32, sg2@64
            psStk = ps.tile([94, W], F32, tag="ps", name="psStk")
            nc.tensor.matmul(psStk[32:62, 0:W], lhsT1x[0:32, 0:C],
                             SF[0:32, 0:W], tile_position=(0, 32))
            nc.tensor.matmul(psStk[64:94, 0:W], lhsT2x[0:32, 0:C],
                             rhs_Z[0:32, 0:W], tile_position=(0, 64))
            # qg2 again at base 0 (for the qg2 . S similarity column)
            psQ2 = ps.tile([C, 1], F32, tag="ps", name="psQ2")
            nc.tensor.matmul(psQ2[:], lhsT2x[0:32, 0:C],
                             rhs_Z[0:32, QC:QC + 1])
            nc.vector.tensor_copy(stk[0:C, 0:W], SF[0:C, 0:W])
            nc.scalar.copy(stk[32:62, 0:W], psStk[32:62, 0:W])
            nc.scalar.copy(stk[64:94, 0:W], psStk[64:94, 0:W])

            # squares for the norms; q-vector columns for the dots
            nc.vector.tensor_mul(sqk[:], stk[:], stk[:])
            for r, base in ((0, 0), (1, 32), (2, 64)):
                nc.vector.tensor_copy(lhsT_q[base:base + C, r:r + 1],
                                      stk[base:base + C, QC:QC + 1])
            nc.vector.tensor_copy(lhsT_q[0:C, 3:4], psQ2[:])

            # dots4 rows: 0 = Sq.S (simA), 1 = qg1.sg1, 2 = qg2.sg2,
            #             3 = qg2.S (simB);  nrm4 rows: |S|,|sg1|,|sg2|,|S|^2
            nrm4 = ps.tile([4, W], F32, tag="ps", name="nrm4")
            nc.tensor.matmul(nrm4[:], mask4[0:94, 0:4], sqk[0:94, 0:W])
            dots4 = ps.tile([4, W], F32, tag="ps", name="dots4")
            nc.tensor.matmul(dots4[:], lhsT_q[0:94, 0:4], stk[0:94, 0:W])

            # q-side squared norms: [|Sq|^2,|qg1|^2,|qg2|^2,|qg2|^2] via G
            qncol = sb.tile([4, 1], F32, tag="qncol", name="qncol")
            nc.scalar.copy(qncol[:], nrm4[0:4, QC:QC + 1])
            psQn = ps.tile([4, 1], F32, tag="ps", name="psQn")
            nc.tensor.matmul(psQn[:], G_sb[:], qncol[:])
            qn = sb.tile([4, 1], F32, tag="qn", name="qn")
            nc.scalar.copy(qn[:], psQn[:])
            ppack = sb.tile([4, W], F32, tag="ppack", name="ppack")
            nc.vector.tensor_scalar(ppack[:], nrm4[0:4, 0:W], qn[:],
                                    None, ALU.mult)

            # rsqrt(ppack) on DVE: magic-number seed + 2 Newton steps
            ish = sb.tile([4, W], U32, tag="ish", name="ish")
            nc.vector.tensor_scalar(ish[:], ppack[:].bitcast(U32), 1,
                                    None, ALU.logical_shift_right)
            y0 = sb.tile([4, W], F32, tag="y0", name="y0")
            nc.vector.tensor_sub(y0[:].bitcast(U32), magic[:], ish[:])
            na = sb.tile([4, W], F32, tag="na", name="na")
            nb = sb.tile([4, W], F32, tag="nb", name="nb")
            nc.vector.tensor_mul(na[:], y0[:], y0[:])
            nc.vector.tensor_mul(nb[:], na[:], ppack[:])
            nc.vector.tensor_scalar(nb[:], nb[:], -0.5, 1.5, ALU.mult,
                                    ALU.add)
            rs = sb.tile([4, W], F32, tag="rs", name="rs")
            nc.vector.tensor_mul(rs[:], y0[:], nb[:])
            # second Newton step on the question-norm column only is not
            # needed: one step leaves ~1.7e-3 relative error on the cosine
            # sims, far inside the 2e-2 gate.

            # sims straight into the MLP lhsT rows 0-3
            nc.vector.tensor_mul(mlp_all[0:4, 0:W], dots4[0:4, 0:W], rs[:])

            # sim1 = simB, except sentence 0 of core 0 -> simA.
            dif_ps = ps.tile([1, 1], F32, tag="ps", name="dif_ps")
            nc.tensor.matmul(dif_ps[:], sdif[0:4, 0:1], mlp_all[0:4, 0:1])
            md = sb.tile([1, 1], F32, tag="md", name="md")
            nc.vector.tensor_mul(md[:], dif_ps[:], mask_sb[:])
            lcorr = sb.tile([1, 2], F32, tag="lcorr", name="lcorr")
            nc.vector.tensor_scalar(lcorr[:], lwr0[:], md[:], None, ALU.mult)

            # ---------------- MLP + 2-class softmax + outputs ------------
            logits = ps.tile([NS, 2], F32, tag="ps", name="logits")
            nc.tensor.matmul(logits[:], mlp_all[0:14, 0:NS], wall_sb[:])
            nc.vector.tensor_add(logits[0:1, 0:2], logits[0:1, 0:2],
                                 lcorr[:])
            lg_sb = sb.tile([NS, 2], F32, tag="lg_sb", name="lg_sb")
            nc.vector.tensor_copy(lg_sb[:], logits[:])
            dz = sb.tile([NS, 1], F32, tag="dz", name="dz")
            nc.vector.tensor_sub(dz[:], lg_sb[:, 1:2], lg_sb[:, 0:1])
            sdz = sb.tile([NS, 1], F32, tag="sdz", name="sdz")
            nc.vector.tensor_mul(sdz[:], dz[:], slab_sb[:])
            # emit = sigmoid(dz); picked log-prob = ln(sigmoid(sdz))
            em = sb.tile([NS, 1], F32, tag="em", name="em")
            nc.scalar.activation(em[:], dz[:], ACTF.Sigmoid)
            psel = sb.tile([NS, 1], F32, tag="psel", name="psel")
            nc.scalar.activation(psel[:], sdz[:], ACTF.Sigmoid)
            pick = sb.tile([NS, 1], F32, tag="pick", name="pick")
            nc.scalar.activation(pick[:], psel[:], ACTF.Ln)
            costp = ps.tile([1, 1], F32, tag="ps", name="costp")
            nc.tensor.matmul(costp[:], ones_sb[:, :], pick[:])
            # partial cost (already scaled); host sums the 8 partials
            nc.vector.tensor_scalar(outst[0:1, NS:NS + 1], costp[:],
                                    -1.0 / N, None, ALU.mult)
            psE = ps.tile([1, NS], F32, tag="ps", name="psE")
            nc.tensor.transpose(psE[:], em[:], iden_sb[:, :])
            nc.vector.tensor_copy(outst[0:1, 0:NS], psE[:])
            nc.sync.dma_start(out[0:1, 0:W], outst[:])

    nc.compile()
    return nc


def _get_nc():
    if "nc" not in _CACHE:
        _CACHE["nc"] = _build()
    return _CACHE["nc"]


def _make_in_maps(question_embeds, sents_embeds, sents_gaf, sents_labels,
                  conv1_w, conv1_b, conv2_w, conv2_b, lin_w, lin_b):
    f32 = lambda a: np.ascontiguousarray(np.asarray(a), dtype=np.float32)
    q = f32(question_embeds)                       # [128, 30]
    x = f32(sents_embeds).reshape(N, S * C)        # [1024, 15360]
    gaf = f32(sents_gaf)                           # [1024, 9]
    lab = f32(np.asarray(sents_labels))            # [1024]
    w1m = f32(conv1_w).reshape(C, C * 4)
    w2m = f32(conv2_w).reshape(C, C * 4)
    b1m = f32(conv1_b).reshape(1, C)
    b2m = f32(conv2_b).reshape(1, C)
    lwm = f32(lin_w)                               # [2, 12]
    lbm = f32(lin_b).reshape(1, 2)
    iden = np.eye(128, dtype=np.float32)
    # question boundary columns, gap rows zeroed to match the stacks
    qbnd = np.zeros((94, 2), dtype=np.float32)
    for t, base in enumerate((0, 32, 64)):
        qbnd[base:base + C, 0] = q[t]              # front: q(0), q(1), q(2)
        qbnd[base:base + C, 1] = q[Q - 3 + t]      # back
    # MLP weights: rows = [0 (simA); w_sim2; w_sim3; w_sim1; gaf(9); bias]
    wallm = np.zeros((14, 2), dtype=np.float32)
    wallm[1] = lwm[:, 1]
    wallm[2] = lwm[:, 2]
    wallm[3] = lwm[:, 0]
    wallm[4:13] = lwm[:, 3:12].T
    wallm[13] = lbm[0]

    in_maps = []
    for k in range(NCORES):
        sl = slice(k * NS, (k + 1) * NS)
        aux = np.zeros((16, W + 1), dtype=np.float32)
        aux[0, 0:NS] = float(S)       # pool output length, sentences (512)
        aux[0, QC] = float(Q)         # pool output length, question (128)
        aux[1, 0:NS] = S + 3.0        # conv output length, sentences (515)
        aux[1, QC] = Q + 3.0          # conv output length, question (131)
        aux[2, 0] = 1.0 if k == 0 else 0.0      # first-sentence mask
        aux[3:12, 0:NS] = gaf[sl].T             # gaf features, transposed
        aux[12, 0:NS] = 1.0                     # ones row (bias feature)
        aux[13, 0:4] = [1.0, 0.0, 0.0, -1.0]    # sdif selector
        # G: picks [|Sq|^2, |qg1|^2, |qg2|^2, |qg2|^2] from the nrm q-col
        G = np.zeros((4, 4), dtype=np.float32)
        G[0, 0] = G[1, 1] = G[2, 2] = G[2, 3] = 1.0
        aux[14, 0:16] = G.reshape(16)
        aux[15, 0:2] = lwm[:, 0]                # sim1 weights (lcorr)
        in_maps.append({
            "xs": np.ascontiguousarray(x[sl]),
            "qe": q,
            "w1": w1m, "b1": b1m, "w2": w2m, "b2": b2m,
            "aux": aux,
            "qbnd": qbnd,
            "slab": (2.0 * np.ascontiguousarray(lab[sl]).reshape(NS, 1)
                     - 1.0).astype(np.float32),
            "wall": wallm,
            "iden": iden,
        })
    return in_maps


def kernel(question_embeds, sents_embeds, sents_gaf, sents_labels,
           conv1_w, conv1_b, conv2_w, conv2_b, lin_w, lin_b):
    in_maps = _make_in_maps(question_embeds, sents_embeds, sents_gaf,
                            sents_labels, conv1_w, conv1_b, conv2_w, conv2_b,
                            lin_w, lin_b)
    nc = _get_nc()
    trace = bool(int(os.environ.get("BASSK_TRACE", "0")))
    res = run_bass_kernel_spmd(nc, in_maps, core_ids=list(range(NCORES)),
                               trace=trace)
    if trace and res.exec_time_ns is not None:
        print(f"HW exec time: {res.exec_time_ns} ns")
        if res.instructions_and_trace is not None:
            print("trace:", res.instructions_and_trace[1])

    emit = np.concatenate([res.results[k]["out"][0, 0:NS]
                           for k in range(NCORES)]).astype(np.float32)
    cost = np.float32(sum(float(res.results[k]["out"][0, NS])
                          for k in range(NCORES)))
    return np.asarray(cost), emit


if __name__ == "__main__":
    _build()
    print("build + compile OK")
